# revision 1
# baseline (speedup 1.0000x reference)
"""Causal self-attention (with head-indexed "RoPE" that is a mathematical
no-op on attention scores) on 8 Trainium2 NeuronCores.

Sharding: 2-way data parallel on batch x 4-way tensor parallel on heads.
Core c = bi*4 + hg owns batches [2*bi, 2*bi+1] and heads [4*hg, 4*hg+4).
Each core computes qkv -> causal attention -> partial c_proj for its head
group; the host sums the 4 partials per batch group and adds b_proj
(the "all-reduce after c_proj" done on host).

Numerics: q/k and the S=q.k^T matmul in bf16; everything else (qkv
accumulation, exp, P.V, projection) in fp32r (hw-rounded fp32, 1
cycle/row like bf16 but ~1.5e-4 relative error). End-to-end relative
error ~2e-4..1e-3.

Key trick: the reference applies RoPE with rotation angles that depend on
the HEAD index, not the position (axis-1 bug faithfully reproduced in the
reference). Since q and k of the same head receive the SAME orthogonal
rotation per 2D pair, q_rot . k_rot == q . k exactly, so attention scores
(and the whole output) are unchanged — RoPE is skipped entirely.

Schedule: engines execute their instruction streams in order, so the
emission order software-pipelines everything: the attention inner loop
emits MM1(kc+1) before exp/MM2(kc), qkv of batch b+1 is woven into the
attention of batch b, and the projection of batch b is woven into the
attention of batch b+1.
"""
import sys

if "/opt/trn_rl_repo" not in sys.path:
    sys.path.insert(0, "/opt/trn_rl_repo")

import numpy as np

N_EMBD = 1024
N_HEAD = 16
HEAD_DIM = 64
B, T = 4, 2048
N_CORES = 8
B_LOC = 2          # batches per core
H_LOC = 4          # heads per core
M_LOC = B_LOC * T  # 4096 rows per core
SCALE = 1.0 / 8.0  # 1/sqrt(64)

_CACHE = {}


def _chunks(q0, hi=1024, step=512):
    """512-aligned column chunks of [q0, hi)."""
    out = []
    c0 = q0
    while c0 < hi:
        c1 = min((c0 // step + 1) * step, hi)
        out.append((c0, c1))
        c0 = c1
    return out


def _build(reps=1):
    import concourse.tile as tile
    from concourse import bacc, mybir

    fr = mybir.dt.float32r
    f32 = mybir.dt.float32
    bf16 = mybir.dt.bfloat16
    AF = mybir.ActivationFunctionType

    nc = bacc.Bacc("TRN2", target_bir_lowering=False, debug=False,
                   num_devices=N_CORES)

    xT = nc.dram_tensor("xT", [N_EMBD, M_LOC], fr, kind="ExternalInput").ap()
    wqk = nc.dram_tensor("wqk", [N_EMBD, 512], fr, kind="ExternalInput").ap()
    wv = nc.dram_tensor("wv", [N_EMBD, 256], fr, kind="ExternalInput").ap()
    wproj = nc.dram_tensor("wproj", [256, 1024], fr, kind="ExternalInput").ap()
    bqk = nc.dram_tensor("bqk", [128, 4], f32, kind="ExternalInput").ap()
    bv = nc.dram_tensor("bv", [128, 256], f32, kind="ExternalInput").ap()
    # multiplicative causal mask: 1.0 where key<=query else 0.0
    mask = nc.dram_tensor("mask", [128, 128], f32, kind="ExternalInput").ap()
    out = nc.dram_tensor("out", [M_LOC, 1024], f32, kind="ExternalOutput").ap()

    MCH = 512                 # m-chunk width for the qkv/v pass
    NMCH = T // MCH           # 4 chunks per batch

    with tile.TileContext(nc) as tc:
        with tc.tile_pool(name="consts", bufs=1) as consts, \
             tc.tile_pool(name="xtp", bufs=2) as xtp, \
             tc.tile_pool(name="qkp", bufs=2) as qkp, \
             tc.tile_pool(name="vap", bufs=2) as vap, \
             tc.tile_pool(name="ptp", bufs=4) as ptp, \
             tc.tile_pool(name="ynp", bufs=2) as ynp, \
             tc.tile_pool(name="smal", bufs=2) as smal, \
             tc.tile_pool(name="outp", bufs=4) as outp, \
             tc.tile_pool(name="ps_s", bufs=2, space="PSUM") as ps_s, \
             tc.tile_pool(name="ps_y", bufs=1, space="PSUM") as ps_y, \
             tc.tile_pool(name="ps_m", bufs=2, space="PSUM") as ps_m:

            wqk_sb = consts.tile([128, 8, 512], fr, tag="wqk", name="wqk_sb")
            wv_sb = consts.tile([128, 8, 256], fr, tag="wv", name="wv_sb")
            wp_sb = consts.tile([128, 2, 1024], fr, tag="wp", name="wp_sb")
            bqk_sb = consts.tile([128, 4], f32, tag="bqk", name="bqk_sb")
            bv_sb = consts.tile([128, 256], f32, tag="bv", name="bv_sb")
            mk_sb = consts.tile([128, 128], f32, tag="mask", name="mk_sb")
            # order: what the first qkv chunk needs comes first, split per
            # 128-row chunk so the first matmuls can start almost immediately
            wqk_r = wqk.rearrange("(c p) n -> p c n", p=128)
            for cc in range(8):
                nc.sync.dma_start(out=wqk_sb[:, cc, :], in_=wqk_r[:, cc, :])
            nc.sync.dma_start(out=bqk_sb, in_=bqk)
            nc.sync.dma_start(out=wv_sb, in_=wv.rearrange("(c p) n -> p c n", p=128))
            nc.sync.dma_start(out=bv_sb, in_=bv)
            # mask + wproj are loaded later (see woven schedule)

            xTr = xT.rearrange("(c p) m -> p c m", p=128)

            state = {}

            def emit_qkv_chunk(rep, b, mch):
                """One 256-wide m-chunk of the qkv + v pass for batch b."""
                if mch == 0:
                    state[(rep, b, "qkT")] = [
                        qkp.tile([128, T], bf16, tag=f"qkT{nt}",
                                 name=f"qkT{nt}_{rep}_{b}")
                        for nt in range(4)]
                    va = vap.tile([128, T // 128, H_LOC, 65], fr, tag="vaug",
                                  name=f"vaug_{rep}_{b}")
                    state[(rep, b, "vaug")] = va
                    # col 64 of each head slot stays 1.0 (softmax denominator)
                    nc.gpsimd.memset(va[:].bitcast(f32), 1.0)
                qkT = state[(rep, b, "qkT")]
                va = state[(rep, b, "vaug")]
                m0 = b * T + mch * MCH
                xt = xtp.tile([128, 8, MCH], fr, tag="xt", name=f"xt_{rep}_{b}_{mch}")
                for cc in range(8):
                    nc.sync.dma_start(out=xt[:, cc, :], in_=xTr[:, cc, m0:m0 + MCH])
                for nt in range(4):
                    ps = ps_m.tile([128, 512], f32, tag="mm", name=f"q_{rep}_{b}_{mch}_{nt}")
                    for cc in range(8):
                        nc.tensor.matmul(ps[:, :MCH],
                                         wqk_sb[:, cc, nt * 128:(nt + 1) * 128],
                                         xt[:, cc, :],
                                         start=(cc == 0), stop=(cc == 7))
                    nc.vector.tensor_scalar_add(
                        out=qkT[nt][:, mch * MCH:(mch + 1) * MCH],
                        in0=ps[:, :MCH], scalar1=bqk_sb[:, nt:nt + 1])
                for mt in range(MCH // 128):
                    psv = ps_m.tile([128, 512], f32, tag="mm",
                                    name=f"v_{rep}_{b}_{mch}_{mt}")
                    for cc in range(8):
                        nc.tensor.matmul(psv[:, :256],
                                         xt[:, cc, mt * 128:(mt + 1) * 128],
                                         wv_sb[:, cc, :],
                                         start=(cc == 0), stop=(cc == 7))
                    kt = mch * (MCH // 128) + mt
                    nc.vector.tensor_add(
                        out=va[:, kt, :, 0:64],
                        in0=psv[:, :256].rearrange("p (h d) -> p h d", h=4),
                        in1=bv_sb.rearrange("p (h d) -> p h d", h=4))

            def emit_attn_unit(rep, b, hp, qq):
                """Attention for head-pair hp (heads 2hp, 2hp+1), query block
                qq (512 queries) of batch b. The two heads' S matmuls run
                concurrently in the top/bottom halves of the PE array; their
                P tiles share one [128, 1024] sbuf tile ([A | B]) so one exp
                covers both."""
                qkT = state[(rep, b, "qkT")]
                va = state[(rep, b, "vaug")]
                if hp == 0:
                    state[(rep, b, "ynT", qq)] = ynp.tile(
                        [128, 2, 512], fr, tag=f"ynT{qq}",
                        name=f"ynT_{rep}_{b}_{qq}")
                ynT = state[(rep, b, "ynT", qq)]
                qt = qkT[hp]
                ktl = qkT[2 + hp]
                hA, hB = 2 * hp, 2 * hp + 1
                # yps: cols 0-511 = head A, 512-1023 = head B
                yps = ps_y.tile([65, 1024], f32, tag="yT",
                                name=f"yT_{rep}_{b}_{hp}_{qq}")
                nkc = 4 * qq + 4
                klast = nkc - 1

                # depth-2 software pipeline: exp trails MM1 by one kc, MM2
                # trails by two, so a unit's first MM2 (which waits on the
                # previous unit's norm to release the ps_y slot) sits behind
                # three MM1s and two exps in the engine streams.
                expq = []   # awaiting exp+mask: (kc, q0, sps)
                mmq = []    # awaiting MM2: (kc, q0, pt)

                def do_exp(kc, q0, sps):
                    pt = ptp.tile([128, 1024], fr, tag="pt",
                                  name=f"pt_{rep}_{b}_{hp}_{qq}_{kc}")
                    nc.scalar.activation(pt[:, q0:1024], sps[:, q0:1024],
                                         AF.Exp, scale=SCALE)
                    if kc >= 4 * qq:
                        nc.vector.tensor_mul(
                            out=pt[:, q0:q0 + 128],
                            in0=pt[:, q0:q0 + 128], in1=mk_sb)
                        nc.vector.tensor_mul(
                            out=pt[:, 512 + q0:512 + q0 + 128],
                            in0=pt[:, 512 + q0:512 + q0 + 128], in1=mk_sb)
                    mmq.append((kc, q0, pt))

                def do_mm2(kc, q0, pt):
                    nc.tensor.matmul(
                        yps[:, q0:512], va[:, kc, hA, :], pt[:, q0:512],
                        start=(kc == 0), stop=(kc == klast))
                    nc.tensor.matmul(
                        yps[:, 512 + q0:1024], va[:, kc, hB, :],
                        pt[:, 512 + q0:1024],
                        start=(kc == 0), stop=(kc == klast))

                for kc in range(nkc):
                    q0 = max(kc * 128 - qq * 512, 0)
                    sps = ps_s.tile([128, 1024], f32, tag="sT",
                                    name=f"sT_{rep}_{b}_{hp}_{qq}_{kc}")
                    nc.tensor.matmul(
                        sps[:, q0:512],
                        ktl[0:64, kc * 128:(kc + 1) * 128],
                        qt[0:64, qq * 512 + q0:(qq + 1) * 512],
                        start=True, stop=True)
                    nc.tensor.matmul(
                        sps[:, 512 + q0:1024],
                        ktl[64:128, kc * 128:(kc + 1) * 128],
                        qt[64:128, qq * 512 + q0:(qq + 1) * 512],
                        start=True, stop=True)
                    if expq:
                        do_exp(*expq.pop(0))
                    if len(mmq) >= 2:
                        do_mm2(*mmq.pop(0))
                    expq.append((kc, q0, sps))
                while expq:
                    do_exp(*expq.pop(0))
                while mmq:
                    do_mm2(*mmq.pop(0))

                # normalize: y / l  (l = row 64 of yps; A and B in one go)
                rt = smal.tile([1, 1024], f32, tag="rt", name=f"rt_{rep}_{b}_{hp}_{qq}")
                nc.vector.reciprocal(rt, yps[64:65, :])
                rb = smal.tile([64, 1024], f32, tag="rb", name=f"rb_{rep}_{b}_{hp}_{qq}")
                nc.gpsimd.partition_broadcast(rb, rt)
                nc.vector.tensor_mul(
                    out=ynT[0:64, hp, 0:512],
                    in0=yps[0:64, 0:512], in1=rb[:, 0:512])
                nc.vector.tensor_mul(
                    out=ynT[64:128, hp, 0:512],
                    in0=yps[0:64, 512:1024], in1=rb[:, 512:1024])

            def emit_proj_mt(rep, b, mt):
                """One 128-row tile of the output projection for batch b."""
                ynT = state[(rep, b, "ynT", mt // 4)]
                for nch in range(2):
                    pso = ps_m.tile([128, 512], f32, tag="mm",
                                    name=f"o_{rep}_{b}_{mt}_{nch}")
                    for hp in range(2):
                        lm = (mt % 4) * 128
                        nc.tensor.matmul(
                            pso,
                            ynT[:, hp, lm:lm + 128],
                            wp_sb[:, hp, nch * 512:(nch + 1) * 512],
                            start=(hp == 0), stop=(hp == 1))
                    ob = outp.tile([128, 512], f32, tag="ob",
                                   name=f"ob_{rep}_{b}_{mt}_{nch}")
                    nc.vector.tensor_copy(out=ob, in_=pso)
                    nc.sync.dma_start(
                        out=out[b * T + mt * 128:b * T + (mt + 1) * 128,
                                nch * 512:(nch + 1) * 512],
                        in_=ob)

            # ---- woven schedule ----
            # b0: hp-major attention with next-batch qkv chunks as PE filler.
            # b1: hp=0 units weave b0's projection (ynT(b0) fully written, no
            # WAR); hp=1 units weave b1's own projection per finished query
            # block (per-qq ynT tiles avoid WAR with later norm writes), so
            # the 8MB output DMA spreads out and there is no projection tail.
            for rep in range(reps):
                for mch in range(NMCH):
                    emit_qkv_chunk(rep, 0, mch)
                    if rep == 0 and mch == 0:
                        nc.sync.dma_start(out=mk_sb, in_=mask)
                    if rep == 0 and mch == 1:
                        nc.sync.dma_start(
                            out=wp_sb,
                            in_=wproj.rearrange("(h p) n -> p h n", p=128))
                for ui, (hp, qq) in enumerate(
                        [(hp, qq) for hp in range(2) for qq in range(4)]):
                    emit_attn_unit(rep, 0, hp, qq)
                    if ui % 2 == 0:
                        emit_qkv_chunk(rep, 1, ui // 2)
                # proj(b0) weave, weighted toward ACT-heavy high-qq units;
                # proj(b1, qq) delayed one unit so it never sits right behind
                # the norm DVE that produces its input.
                pb0 = {0: [0, 1], 1: [2, 3, 4], 2: [5, 6, 7], 3: [8, 9, 10]}
                for qq in range(4):
                    emit_attn_unit(rep, 1, 0, qq)
                    for mt in pb0.get(qq, []):
                        emit_proj_mt(rep, 0, mt)
                    if rep + 1 < reps:
                        emit_qkv_chunk(rep + 1, 0, qq)
                for qq in range(4):
                    emit_attn_unit(rep, 1, 1, qq)
                    for mt in ([11, 12] if qq == 0 else [13, 14, 15] if qq == 1 else []):
                        emit_proj_mt(rep, 0, mt)
                    if qq > 0:
                        for mt in range(4 * (qq - 1), 4 * qq):
                            emit_proj_mt(rep, 1, mt)
                for mt in range(12, 16):
                    emit_proj_mt(rep, 1, mt)
    nc.compile()
    return nc


def _get_nc(reps=1):
    key = f"nc{reps}"
    if key not in _CACHE:
        _CACHE[key] = _build(reps)
    return _CACHE[key]


def kernel(x, W_attn, b_attn, W_proj, b_proj):
    from concourse import bass_utils

    x = np.asarray(x, dtype=np.float32)
    W_attn = np.asarray(W_attn, dtype=np.float32)
    b_attn = np.asarray(b_attn, dtype=np.float32)
    W_proj = np.asarray(W_proj, dtype=np.float32)
    b_proj = np.asarray(b_proj, dtype=np.float32)

    Wq, Wk, Wv = W_attn[:, :1024], W_attn[:, 1024:2048], W_attn[:, 2048:]
    bq, bk, bv = b_attn[:1024], b_attn[1024:2048], b_attn[2048:]

    kp = np.arange(128)[:, None]
    qf = np.arange(128)[None, :]
    mask = (kp <= qf).astype(np.float32)

    in_maps = []
    for core in range(N_CORES):
        bi, hg = core // 4, core % 4
        s = slice(256 * hg, 256 * hg + 256)
        xT = np.ascontiguousarray(
            x[2 * bi:2 * bi + 2].reshape(M_LOC, N_EMBD).T)
        wqk = np.ascontiguousarray(np.concatenate([Wq[:, s], Wk[:, s]], axis=1))
        bqk = np.ascontiguousarray(
            np.concatenate([bq[s], bk[s]]).reshape(4, 128).T)
        in_maps.append({
            "xT": xT,
            "wqk": wqk,
            "wv": np.ascontiguousarray(Wv[:, s]),
            "wproj": np.ascontiguousarray(W_proj[s, :]),
            "bqk": bqk,
            "bv": np.ascontiguousarray(np.broadcast_to(bv[s], (128, 256))),
            "mask": mask,
        })

    nc = _get_nc()
    _CACHE["last_in_maps"] = in_maps
    res = bass_utils.run_bass_kernel_spmd(nc, in_maps, core_ids=list(range(N_CORES)))

    full = np.zeros((B, T, N_EMBD), dtype=np.float32)
    for bi in range(2):
        acc = np.zeros((M_LOC, N_EMBD), dtype=np.float32)
        for hg in range(4):
            acc += res.results[bi * 4 + hg]["out"]
        full[2 * bi:2 * bi + 2] = acc.reshape(B_LOC, T, N_EMBD)
    full += b_proj
    return full

